# revision 33
# baseline (speedup 1.0000x reference)
"""Self-contained Trainium2 Bass kernel for nn_Classifier_79929341379065.

kernel(**inputs) takes FULL unsharded inputs (as produced by
reference.setup_inputs()) and returns the FULL [B, 1] float32 output.
Internally: pure data parallel over 8 NeuronCores (batch dim of x),
weights replicated.

Hardcoded shapes: B=8192, L=16, H=8, DK=DV=32, DM=256, BN=128, V=50000.
Per core: 1024 batches = 16384 tokens = 128 subtiles of 128 tokens
(each subtile = 8 attention groups of L=16), processed in supertiles of
4 subtiles (512 tokens) so matmuls stream 512 columns.

v2 design (instruction-count-minimized; matmul ~= 165ns + 0.73ns/col):
  host folds LN1/LN2 + 1/sqrt(DK) into a per-head bilinear form
     A_h = (Wq_eff_h)^T (Wk_eff_h)  [BN x BN]  (stored as lhsT)
  per supertile: gather [n|em] -> 8 PE transposes -> neT (nT|emT wide)
  G_h = A_h @ nT    (8 matmuls x 512 cols)  replaces q4+k entirely
  per subtile: S^T[t_k,(h,t_q)] = nT_s^T @ G  (2 x 512, one stationary)
  exp -> *mask (vector) -> PV per head with ones-aug v (den fused)
  ctx normalize -> 2 transposes -> ctxT wide
  MLP feature-major, 512-wide: fc1 4x512, h1 4x512, h2 2x512;
  u (residual) via transpose-matmuls + p1w2 accumulated in one PSUM
  bank per subtile; st per subtile 2x256.  LN tails as before; the
  dd/dw/wsq head chain on vector (gpsimd tensor_tensor is 2.2us/op).
  Supertile tails are emitted AFTER the next supertile's attention head
  so the PE queue never drains while vector works the tail.
"""

import os
import sys
import types

import numpy as np

# ---------------------------------------------------------------- constants
B, L = 8192, 16
H, DK, DV = 8, 32, 32
DM, BN, V = 256, 128, 50000
NCORES = 8
P = 128
BC = B // NCORES                  # batches per core (1024)
TOKC = BC * L                     # tokens per core (16384)
NSUB_FULL = TOKC // P             # subtiles per core (128)
GRP = P // L                      # groups per subtile (8)
ST = 4                            # subtiles per supertile
SW = ST * P                       # supertile width in tokens (512)
SCL = 1.0 / np.sqrt(float(DK))
EPS = 1e-5


def _install_ntff_hook():
    """Register the axon NTFF profiling hook if the image's antenv lacks it,
    so run_bass_kernel_spmd(trace=True) works in this container."""
    try:
        import antenv.axon_hooks  # noqa: F401
        return
    except ImportError:
        pass
    try:
        from trn_agent_boot.trn_boot import _ntff_profile_via_ctypes
        hook = _ntff_profile_via_ctypes("/opt/axon/libaxon_pjrt.so")
    except Exception:
        hook = None
    m = types.ModuleType("antenv.axon_hooks")
    m.get_axon_ntff_profile_hook = lambda: hook
    m.set_axon_ntff_profile_hook = lambda h: None
    sys.modules["antenv.axon_hooks"] = m


def _bf16(a):
    import ml_dtypes
    return np.ascontiguousarray(a.astype(ml_dtypes.bfloat16))


def _triv(g, b):
    return bool(np.allclose(g, 1.0, atol=1e-12) and np.allclose(b, 0.0, atol=1e-12))


# ------------------------------------------------------------- host weights
def _prep_consts(w):
    """Fold LN affines into projection weights; build device const arrays."""
    c = {}
    f32 = np.float32

    wq_eff = (np.asarray(w["Wq"], f32) * np.asarray(w["ln1_g"], f32)[None, :]) * SCL
    wk_eff = np.asarray(w["Wk"], f32) * np.asarray(w["ln2_g"], f32)[None, :]
    wv_eff = np.asarray(w["Wv"], f32) * np.asarray(w["ln3_g"], f32)[None, :]
    cq = (np.asarray(w["ln1_b"], f32) @ np.asarray(w["Wq"], f32).T) * SCL
    ck = np.asarray(w["ln2_b"], f32) @ np.asarray(w["Wk"], f32).T
    cv = np.asarray(w["ln3_b"], f32) @ np.asarray(w["Wv"], f32).T

    # bilinear attention: S^T_h = nT^T A_h nT with A_h = Wk_h^T Wq_h.
    # a8 block h is the lhsT for G_h = A_h @ nT: lhsT[b2,b1]=A_h[b1,b2]
    #   = (wq_eff_h^T wk_eff_h)[b2,b1]
    a8 = np.zeros((BN, H * BN), f32)
    for h in range(H):
        sl = slice(h * DK, (h + 1) * DK)
        a8[:, h * BN:(h + 1) * BN] = wq_eff[sl, :].T @ wk_eff[sl, :]
    c["a8"] = _bf16(a8)

    wv_aug = np.zeros((BN, H * (DV + 1)), f32)               # [128,264]
    for h in range(H):
        wv_aug[:, h * 33:h * 33 + 32] = wv_eff.T[:, h * 32:(h + 1) * 32]
    c["wv"] = _bf16(wv_aug)

    c["wfc1"] = _bf16(np.asarray(w["Wfc1"], f32).T)          # [HDV, DM]
    c["p1w1"] = _bf16(np.asarray(w["p1_w1"], f32).T)         # [DM, DM]
    c["p1w2"] = _bf16(np.asarray(w["p1_w2"], f32).T)         # [DM, DM]
    c["p2w1"] = _bf16(np.asarray(w["p2_w1"], f32).T)         # [BN, DM]
    c["p2w2"] = _bf16(np.asarray(w["p2_w2"], f32).T)         # [DM, DM]

    # merged gather table: [V, 256] = [LN-normalized | raw with row0 zeroed]
    tab = np.asarray(w["node_emb"], f32)
    m = tab.mean(axis=1, keepdims=True)
    v = ((tab - m) ** 2).mean(axis=1, keepdims=True)
    tabn = (tab - m) / np.sqrt(v + EPS)
    tabe = tab.copy()
    tabe[0, :] = 0.0
    c["tabs"] = _bf16(np.concatenate([tabn, tabe], axis=1))  # [V, 256]

    # block-diag(16)-minus-eye multiplicative mask, tiled 8x (8 head slots)
    blk = np.zeros((P, P), f32)
    for g in range(GRP):
        blk[g * L:(g + 1) * L, g * L:(g + 1) * L] = 1.0
    blk -= np.eye(P, dtype=f32)
    blk = np.maximum(blk, 0.0)
    c["mask8"] = _bf16(np.tile(blk, (1, 8)))                 # [128,1024]

    gind = np.zeros((P, GRP), f32)
    for g in range(GRP):
        gind[g * L:(g + 1) * L, g] = 1.0
    c["gind"] = gind

    wcls_row = np.asarray(w["Wcls"], f32).reshape(1, DM)
    c["wcls4"] = np.ascontiguousarray(
        np.broadcast_to(np.tile(wcls_row, (1, ST)), (P, ST * DM)))  # [128,1024]
    c["ident"] = _bf16(np.eye(P, dtype=f32))

    flags = {
        "qkb": not (np.allclose(cq, 0.0) and np.allclose(ck, 0.0)),
        "cv": not np.allclose(cv, 0.0),
        "p1b1": not np.allclose(w["p1_b1"], 0.0),
        "p2b1": not np.allclose(w["p2_b1"], 0.0),
        "p1b2": not np.allclose(w["p1_b2"], 0.0),
        "p2b2": not np.allclose(w["p2_b2"], 0.0),
        "p1aff": not _triv(w["p1_lng"], w["p1_lnb"]),
        "c1aff": not _triv(w["lnc1_g"], w["lnc1_b"]),
        "c2aff": not _triv(w["lnc2_g"], w["lnc2_b"]),
    }
    flags["lnc1"] = flags["p1aff"]

    if flags["qkb"]:
        # S gains + (wk_eff_h^T cq_h) . n_tk  per (t_k, h); the t_q-side and
        # constant terms cancel in softmax. Applied as ptm *= exp(c)[tk,h].
        wc = np.zeros((BN, H), f32)
        for h in range(H):
            sl = slice(h * DK, (h + 1) * DK)
            wc[:, h] = wk_eff[sl, :].T @ cq[sl]
        c["wc"] = _bf16(wc)
    if flags["cv"]:
        cvb = np.zeros((P, H * 33), f32)
        for h in range(H):
            cvb[:, h * 33:h * 33 + 32] = np.broadcast_to(
                cv[h * 32:(h + 1) * 32][None, :], (P, 32))
        c["cvb"] = cvb
    if flags["p1b1"]:
        c["p1b1"] = np.stack([np.asarray(w["p1_b1"], f32)[0:128],
                              np.asarray(w["p1_b1"], f32)[128:256]], 1)
    if flags["p2b1"]:
        c["p2b1"] = np.stack([np.asarray(w["p2_b1"], f32)[0:128],
                              np.asarray(w["p2_b1"], f32)[128:256]], 1)
    if flags["p1b2"]:
        c["p1b2b"] = np.broadcast_to(
            np.asarray(w["p1_b2"], f32)[None, :], (P, DM)).copy()
    if flags["p2b2"]:
        c["p2b2b"] = np.broadcast_to(
            np.asarray(w["p2_b2"], f32)[None, :], (P, DM)).copy()
    for nm, gk, bk in (("p1", "p1_lng", "p1_lnb"), ("c1", "lnc1_g", "lnc1_b"),
                       ("c2", "lnc2_g", "lnc2_b")):
        if flags[nm + "aff"]:
            c[nm + "gb"] = np.broadcast_to(
                np.asarray(w[gk], f32)[None, :], (P, DM)).copy()
            c[nm + "bb"] = np.broadcast_to(
                np.asarray(w[bk], f32)[None, :], (P, DM)).copy()

    c["_bcls"] = float(np.asarray(w["bcls"]).reshape(-1)[0])
    c["_flags"] = flags
    return c


# ------------------------------------------------------------ device program
def build_nc(flags, bcls, n_sub):
    import contextlib

    import concourse.bacc as bacc
    import concourse.tile as tile
    import concourse.mybir as mybir
    from concourse import bass

    dt = mybir.dt
    AF = mybir.ActivationFunctionType
    OP = mybir.AluOpType
    IOA = bass.IndirectOffsetOnAxis
    assert n_sub % ST == 0
    nss = n_sub // ST

    nc = bacc.Bacc()

    # ---- dram tensors
    idxc = nc.dram_tensor("idxc", [P, n_sub], dt.int32, kind="ExternalInput")
    npmc = nc.dram_tensor("npmc", [P, n_sub], dt.float32, kind="ExternalInput")
    tabs_d = nc.dram_tensor("tabs", [V, 2 * BN], dt.bfloat16, kind="ExternalInput")
    a8_d = nc.dram_tensor("a8", [BN, H * BN], dt.bfloat16, kind="ExternalInput")
    wv_d = nc.dram_tensor("wv", [BN, 264], dt.bfloat16, kind="ExternalInput")
    wfc1_d = nc.dram_tensor("wfc1", [DM, DM], dt.bfloat16, kind="ExternalInput")
    p1w1_d = nc.dram_tensor("p1w1", [DM, DM], dt.bfloat16, kind="ExternalInput")
    p1w2_d = nc.dram_tensor("p1w2", [DM, DM], dt.bfloat16, kind="ExternalInput")
    p2w1_d = nc.dram_tensor("p2w1", [BN, DM], dt.bfloat16, kind="ExternalInput")
    p2w2_d = nc.dram_tensor("p2w2", [DM, DM], dt.bfloat16, kind="ExternalInput")
    mask_d = nc.dram_tensor("mask8", [P, 1024], dt.bfloat16, kind="ExternalInput")
    gind_d = nc.dram_tensor("gind", [P, GRP], dt.float32, kind="ExternalInput")
    wcls_d = nc.dram_tensor("wcls4", [P, ST * DM], dt.float32, kind="ExternalInput")
    ident_d = nc.dram_tensor("ident", [P, P], dt.bfloat16, kind="ExternalInput")
    opt_d = {}
    for nm, shp, dtp, cond in [
        ("wc", [BN, H], dt.bfloat16, flags["qkb"]),
        ("cvb", [P, 264], dt.float32, flags["cv"]),
        ("p1b1", [P, 2], dt.float32, flags["p1b1"]),
        ("p2b1", [P, 2], dt.float32, flags["p2b1"]),
        ("p1b2b", [P, DM], dt.float32, flags["p1b2"]),
        ("p2b2b", [P, DM], dt.float32, flags["p2b2"]),
        ("p1gb", [P, DM], dt.float32, flags["p1aff"]),
        ("p1bb", [P, DM], dt.float32, flags["p1aff"]),
        ("c1gb", [P, DM], dt.float32, flags["c1aff"]),
        ("c1bb", [P, DM], dt.float32, flags["c1aff"]),
        ("c2gb", [P, DM], dt.float32, flags["c2aff"]),
        ("c2bb", [P, DM], dt.float32, flags["c2aff"]),
    ]:
        if cond:
            opt_d[nm] = nc.dram_tensor(nm, shp, dtp, kind="ExternalInput")
    outp = nc.dram_tensor("outp", [GRP, n_sub], dt.float32, kind="ExternalOutput")
    dbg = int(os.environ.get("KBENCH_DEBUG", "0"))
    dbg_d = {}
    if dbg:
        bf = dt.bfloat16
        f32 = dt.float32
        for nm, shp, dtp in [("d_neT", [P, 2 * SW], bf), ("d_gall", [P, H * SW], bf),
                             ("d_pt", [P, 1024], bf), ("d_ptm", [P, 1024], bf),
                             ("d_ctx", [P, 256], bf), ("d_ctxT", [P, 2 * SW], bf),
                             ("d_din", [P, 2 * SW], bf), ("d_h1", [P, 2 * SW], bf),
                             ("d_h2", [P, 2 * SW], bf), ("d_u2", [P, ST * DM], f32),
                             ("d_st2", [P, ST * DM], f32), ("d_dyn", [P, ST * DM], f32),
                             ("d_sta", [P, ST * DM], f32)]:
            dbg_d[nm] = nc.dram_tensor(nm, shp, dtp, kind="ExternalOutput")

    with tile.TileContext(nc) as tc:
        with contextlib.ExitStack() as ctx:
            singles = ctx.enter_context(tc.tile_pool(name="singles", bufs=1))
            io = ctx.enter_context(tc.tile_pool(name="io", bufs=6))
            work = ctx.enter_context(tc.tile_pool(name="work", bufs=2))
            wsub = ctx.enter_context(tc.tile_pool(name="wsub", bufs=3))
            # PSUM is bank-granular: 8 banks of [128, 2KB]. tr 2 + g 2 + med 4.
            ps_tr = ctx.enter_context(tc.tile_pool(name="ps_tr", bufs=2,
                                                   space="PSUM"))
            ps_g = ctx.enter_context(tc.tile_pool(name="ps_g", bufs=3,
                                                  space="PSUM"))
            ps_m = ctx.enter_context(tc.tile_pool(name="ps_m", bufs=3,
                                                  space="PSUM"))

            def load(d, shape, dtp):
                t = singles.tile(shape, dtp, name=d.name + "_sb")
                nc.sync.dma_start(t[:], d[:, :])
                return t

            idx_sb = load(idxc, [P, n_sub], dt.int32)
            npm_sb = load(npmc, [P, n_sub], dt.float32)
            a8 = load(a8_d, [BN, H * BN], dt.bfloat16)
            wv = load(wv_d, [BN, 264], dt.bfloat16)
            mask_sb = load(mask_d, [P, 1024], dt.bfloat16)
            gind_sb = load(gind_d, [P, GRP], dt.float32)
            wcls_sb = load(wcls_d, [P, ST * DM], dt.float32)
            ident = load(ident_d, [P, P], dt.bfloat16)
            wfc1, p1w1, p1w2, p2w2 = ([None, None] for _ in range(4))
            for k in range(2):
                for nm, arr, d in (("wfc1", wfc1, wfc1_d), ("p1w1", p1w1, p1w1_d),
                                   ("p1w2", p1w2, p1w2_d), ("p2w2", p2w2, p2w2_d)):
                    arr[k] = singles.tile([P, DM], dt.bfloat16, name=f"{nm}_{k}")
                    nc.sync.dma_start(arr[k][:], d[k * P:(k + 1) * P, :])
            p2w1 = load(p2w1_d, [BN, DM], dt.bfloat16)
            osb = {nm: load(d, d.shape,
                            dt.bfloat16 if nm == "wc" else dt.float32)
                   for nm, d in opt_d.items()}

            epst = singles.tile([P, 1], dt.float32, name="epst")
            nc.vector.memset(epst[:], EPS)
            res = singles.tile([GRP, 2 * n_sub], dt.float32, name="res")

            def dump(nm, ap):
                if dbg and nm in dbg_d:
                    nc.sync.dma_start(dbg_d[nm][:, :], ap)
                    dbg_d.pop(nm)

            def emit_tail(ss, mv_st, u2_st, st2_st):
                # batched sqrt/recip of the 8 variances (cols 1,3,5,...,15)
                mvv = mv_st[:].rearrange("p (j k) -> p j k", k=2)
                std_st = wsub.tile([P, 2 * ST], dt.float32, tag="stds",
                                   name="std_st")
                std3 = std_st[:].rearrange("p (j o) -> p j o", o=1)
                nc.scalar.activation(std3[:], mvv[:, :, 1:2], AF.Sqrt,
                                     bias=epst[:, 0:1])
                rstd_st = wsub.tile([P, 2 * ST], dt.float32, tag="rstds",
                                    name="rstd_st")
                nc.vector.reciprocal(rstd_st[:], std_st[:])

                dyn_st = wsub.tile([P, ST * DM], dt.float32, tag="dyn",
                                   name="dyn_st")
                sta_st = wsub.tile([P, ST * DM], dt.float32, tag="sta",
                                   name="sta_st")
                for s in range(ST):
                    dyn_sl = dyn_st[:, s * DM:(s + 1) * DM]
                    nc.vector.tensor_scalar(
                        out=dyn_sl, in0=u2_st[:, s * DM:(s + 1) * DM],
                        scalar1=mv_st[:, 4 * s:4 * s + 1],
                        scalar2=rstd_st[:, 2 * s:2 * s + 1],
                        op0=OP.subtract, op1=OP.mult)
                    if flags["p1aff"]:
                        nc.vector.tensor_mul(dyn_sl, dyn_sl, osb["p1gb"][:])
                        nc.vector.tensor_add(dyn_sl, dyn_sl, osb["p1bb"][:])
                        nc.vector.tensor_scalar_mul(dyn_sl, dyn_sl,
                                                    npm_sb[:, ss * ST + s:
                                                           ss * ST + s + 1])
                    if flags["lnc1"]:
                        st6c = wsub.tile([P, 6], dt.float32, tag="st6c",
                                         name="st6c")
                        nc.vector.bn_stats(st6c[:], dyn_sl)
                        mvc = wsub.tile([P, 2], dt.float32, tag="mvc", name="mvc")
                        nc.vector.bn_aggr(mvc[:], st6c[:])
                        stdc = wsub.tile([P, 1], dt.float32, tag="stdc",
                                         name="stdc")
                        nc.scalar.activation(stdc[:], mvc[:, 1:2], AF.Sqrt,
                                             bias=epst[:, 0:1])
                        rstdc = wsub.tile([P, 1], dt.float32, tag="rstdc",
                                          name="rstdc")
                        nc.vector.reciprocal(rstdc[:], stdc[:])
                        nc.vector.tensor_scalar(
                            out=dyn_sl, in0=dyn_sl, scalar1=mvc[:, 0:1],
                            scalar2=rstdc[:, 0:1], op0=OP.subtract, op1=OP.mult)
                    if flags["c1aff"]:
                        nc.vector.tensor_mul(dyn_sl, dyn_sl, osb["c1gb"][:])
                        nc.vector.tensor_add(dyn_sl, dyn_sl, osb["c1bb"][:])

                    sta_sl = sta_st[:, s * DM:(s + 1) * DM]
                    nc.vector.tensor_scalar(
                        out=sta_sl, in0=st2_st[:, s * DM:(s + 1) * DM],
                        scalar1=mv_st[:, 4 * s + 2:4 * s + 3],
                        scalar2=rstd_st[:, 2 * s + 1:2 * s + 2],
                        op0=OP.subtract, op1=OP.mult)
                    if flags["c2aff"]:
                        nc.vector.tensor_mul(sta_sl, sta_sl, osb["c2gb"][:])
                        nc.vector.tensor_add(sta_sl, sta_sl, osb["c2bb"][:])

                if ss == 0:
                    dump("d_dyn", dyn_st[:])
                    dump("d_sta", sta_st[:])
                # head: logit = sum((dyn-sta)^2 * wcls) ; sigmoid via exp
                dd_st = wsub.tile([P, ST * DM], dt.float32, tag="dd",
                                  name="dd_st")
                nc.gpsimd.tensor_tensor(out=dd_st[:], in0=dyn_st[:],
                                        in1=sta_st[:], op=OP.subtract)
                dw_st = wsub.tile([P, ST * DM], dt.float32, tag="dw",
                                  name="dw_st")
                nc.gpsimd.tensor_tensor(out=dw_st[:], in0=dd_st[:],
                                        in1=wcls_sb[:], op=OP.mult)
                wsq_st = wsub.tile([P, ST * DM], dt.float32, tag="wsq",
                                   name="wsq_st")
                nc.gpsimd.tensor_mul(wsq_st[:], dd_st[:], dw_st[:])
                return ss, wsq_st

            def emit_tail_fin(ss, wsq_st):
                logit_st = wsub.tile([P, ST], dt.float32, tag="lg",
                                     name="logit_st")
                nc.vector.tensor_reduce(
                    logit_st[:], wsq_st[:].rearrange("p (s d) -> p s d", d=DM),
                    axis=mybir.AxisListType.X, op=OP.add)

                e_st = wsub.tile([P, ST], dt.float32, tag="est", name="e_st")
                nc.scalar.activation(e_st[:], logit_st[:], AF.Exp, bias=-bcls,
                                     scale=-1.0)
                pe1 = wsub.tile([P, ST], dt.float32, tag="pe1", name="pe1")
                nc.vector.tensor_scalar_add(pe1[:], e_st[:], 1.0)
                probs_st = wsub.tile([P, ST], dt.float32, tag="pb",
                                     name="probs_st")
                nc.vector.reciprocal(probs_st[:], pe1[:])

                npm4 = npm_sb[:, ss * ST:(ss + 1) * ST]
                pn_st = wsub.tile([P, 2 * ST], dt.float32, tag="pn", name="pn_st")
                pnv = pn_st[:].rearrange("p (s k) -> p s k", k=2)
                nc.vector.tensor_tensor(
                    out=pnv[:, :, 0:1],
                    in0=probs_st[:].rearrange("p (s o) -> p s o", o=1),
                    in1=npm4.rearrange("p (s o) -> p s o", o=1), op=OP.mult)
                nc.gpsimd.tensor_copy(pnv[:, :, 1:2],
                                      npm4.rearrange("p (s o) -> p s o", o=1))

                agg_ps = ps_m.tile([GRP, 2 * ST], dt.float32, tag="med",
                                   name="agg_ps")
                nc.tensor.matmul(agg_ps[:], lhsT=gind_sb[:], rhs=pn_st[:])
                nc.scalar.activation(res[0:GRP, 2 * ST * ss:2 * ST * (ss + 1)],
                                     agg_ps[:], AF.Copy)

            def emit_gathers(ss):
                nes = []
                for s in range(ST):
                    t = ss * ST + s
                    ne = io.tile([P, 2 * BN], dt.bfloat16, tag="ne", name="ne")
                    nc.gpsimd.indirect_dma_start(
                        out=ne[:], out_offset=None, in_=tabs_d[:, :],
                        in_offset=IOA(ap=idx_sb[:, t:t + 1], axis=0))
                    nes.append(ne)
                return nes

            def emit_head(ss, nes):
                neT_st = work.tile([P, 2 * SW], dt.bfloat16, tag="neT",
                                   name="neT_st")
                neT3 = neT_st[:].rearrange("p (b t) -> p b t", b=2)
                for sp in range(ST // 2):
                    ne_ps = ps_tr.tile([P, 4 * P], dt.bfloat16, tag="tr",
                                       name="ne_ps")
                    for hf in range(2):
                        ne = nes[sp * 2 + hf]
                        nc.tensor.transpose(ne_ps[:, hf * P:(hf + 1) * P],
                                            ne[:, 0:P], ident[:])
                        nc.tensor.transpose(
                            ne_ps[:, 2 * P + hf * P:2 * P + (hf + 1) * P],
                            ne[:, P:2 * P], ident[:])
                    nc.scalar.activation(
                        neT3[:, :, sp * 2 * P:(sp + 1) * 2 * P],
                        ne_ps[:].rearrange("p (b t) -> p b t", t=2 * P), AF.Copy)
                dump("d_neT", neT_st[:])

                # G_all layout: [p, (s:4) x (h:8) x 128] so S rhs is flat
                g_all = work.tile([P, H * SW], dt.bfloat16, tag="gall",
                                  name="g_all")
                ga3 = g_all[:].rearrange("p (s h t) -> p s (h t)", s=ST, t=P)
                for h in range(H):
                    g_ps = ps_g.tile([P, SW], dt.float32, tag="g", name="g_ps")
                    nc.tensor.matmul(g_ps[:], lhsT=a8[:, h * P:(h + 1) * P],
                                     rhs=neT_st[:, 0:SW])
                    dst = ga3[:, :, h * P:(h + 1) * P]
                    src = g_ps[:].rearrange("p (s t) -> p s t", t=P)
                    if h % 2 == 0:
                        nc.scalar.activation(dst, src, AF.Copy)
                    else:
                        nc.vector.tensor_copy(dst, src)

                # v for all subtiles (fills PE while G copies drain)
                v_augs = []
                for s in range(ST):
                    v_ps = ps_m.tile([P, 264], dt.float32, tag="med", name="v_ps")
                    nc.tensor.matmul(v_ps[:], lhsT=neT_st[:, s * P:(s + 1) * P],
                                     rhs=wv[:])
                    v_aug = wsub.tile([P, 264], dt.bfloat16, tag=f"va{s}",
                                      name="v_aug")
                    nc.scalar.activation(v_aug[:], v_ps[:], AF.Copy)
                    va3 = v_aug[:].rearrange("p (h c) -> p h c", c=33)
                    if flags["cv"]:
                        nc.vector.tensor_add(v_aug[:], v_aug[:], osb["cvb"][:])
                    nc.gpsimd.memset(va3[:, :, 32:33], 1.0)
                    v_augs.append(v_aug)
                dump("d_gall", g_all[:])
                return neT_st, g_all, v_augs

            def emit_attn(ss, neT_st, g_all, v_augs):
                """S / PV / ctxT software-pipelined so the in-order PE queue
                always has independent work behind each dependency wait."""
                ctxT_st = work.tile([P, 2 * SW], dt.bfloat16, tag="ctxT",
                                    name="ctxT_st")
                cT3 = ctxT_st[:].rearrange("p (b t) -> p b t", b=2)
                ptms = [None] * ST
                ctxs = [None] * ST
                ctps = [None] * (ST // 2)

                def do_S(s):
                    nT = neT_st[:, s * P:(s + 1) * P]
                    pt = wsub.tile([P, 1024], dt.bfloat16, tag="pt", name="pt")
                    for b2 in range(2):
                        s_ps = ps_g.tile([P, 512], dt.float32, tag="g",
                                         name="s_ps")
                        nc.tensor.matmul(
                            s_ps[:], lhsT=nT,
                            rhs=g_all[:, s * 1024 + b2 * 512:
                                      s * 1024 + (b2 + 1) * 512])
                        nc.scalar.activation(pt[:, b2 * 512:(b2 + 1) * 512],
                                             s_ps[:], AF.Exp)
                    if ss == 0 and s == 0:
                        dump("d_pt", pt[:])
                    ptm = wsub.tile([P, 1024], dt.bfloat16, tag="ptm",
                                    name="ptm")
                    nc.vector.tensor_tensor(out=ptm[:], in0=pt[:],
                                            in1=mask_sb[:], op=OP.mult)
                    if flags["qkb"]:
                        cm_ps = ps_m.tile([P, H], dt.float32, tag="med",
                                          name="cm_ps")
                        nc.tensor.matmul(cm_ps[:], lhsT=nT, rhs=osb["wc"][:])
                        expc = wsub.tile([P, H], dt.float32, tag="expc",
                                         name="expc")
                        nc.scalar.activation(expc[:], cm_ps[:], AF.Exp)
                        ptm3 = ptm[:].rearrange("p (h t) -> p h t", t=P)
                        nc.vector.tensor_tensor(
                            out=ptm3[:], in0=ptm3[:],
                            in1=expc[:].rearrange("p (h o) -> p h o", o=1)
                            .to_broadcast([P, H, P]), op=OP.mult)
                    if ss == 0 and s == 0:
                        dump("d_ptm", ptm[:])
                    ptms[s] = ptm

                def do_PV(s):
                    ca_ps = ps_m.tile([P, 264], dt.float32, tag="med",
                                      name="ca_ps")
                    for h in range(H):
                        nc.tensor.matmul(
                            ca_ps[:, h * 33:(h + 1) * 33],
                            lhsT=ptms[s][:, h * P:(h + 1) * P],
                            rhs=v_augs[s][:, h * 33:(h + 1) * 33])
                    ca3 = ca_ps[:].rearrange("p (h c) -> p h c", c=33)
                    rec = wsub.tile([P, H], dt.float32, tag="rec", name="rec")
                    rec3 = rec[:].rearrange("p (h o) -> p h o", o=1)
                    nc.vector.reciprocal(rec3[:], ca3[:, :, 32:33])
                    ctx_bf = wsub.tile([P, 256], dt.bfloat16, tag=f"cx{s % 2}",
                                       name="ctx_bf")
                    cb3 = ctx_bf[:].rearrange("p (h c) -> p h c", c=32)
                    nc.vector.tensor_tensor(out=cb3[:], in0=ca3[:, :, 0:32],
                                            in1=rec3.to_broadcast([P, H, 32]),
                                            op=OP.mult)
                    if ss == 0 and s == 0:
                        dump("d_ctx", ctx_bf[:])
                    ctxs[s] = ctx_bf

                def do_ctxT(s):
                    hf = s % 2
                    if hf == 0:
                        ctps[s // 2] = ps_tr.tile([P, 4 * P], dt.bfloat16,
                                                  tag="tr", name="ct_ps")
                    ct_ps = ctps[s // 2]
                    nc.tensor.transpose(ct_ps[:, hf * P:(hf + 1) * P],
                                        ctxs[s][:, 0:P], ident[:])
                    nc.tensor.transpose(
                        ct_ps[:, 2 * P + hf * P:2 * P + (hf + 1) * P],
                        ctxs[s][:, P:2 * P], ident[:])
                    if hf == 1:
                        sp = s // 2
                        nc.scalar.activation(
                            cT3[:, :, sp * 2 * P:(sp + 1) * 2 * P],
                            ct_ps[:].rearrange("p (b t) -> p b t", t=2 * P),
                            AF.Copy)

                do_S(0)
                do_S(1)
                do_PV(0)
                do_S(2)
                do_PV(1)
                do_ctxT(0)
                do_S(3)
                do_PV(2)
                do_ctxT(1)
                do_PV(3)
                do_ctxT(2)
                do_ctxT(3)
                dump("d_ctxT", ctxT_st[:])
                return ctxT_st

            def emit_mlp(ss, neT_st, ctxT_st):
                mv_st = wsub.tile([P, 4 * ST], dt.float32, tag="mv", name="mv_st")
                u2_st = wsub.tile([P, ST * DM], dt.float32, tag="u2",
                                  name="u2_st")
                st2_st = wsub.tile([P, ST * DM], dt.float32, tag="st2",
                                   name="st2_st")
                din_sb = work.tile([P, 2 * SW], dt.bfloat16, tag="din",
                                   name="din_sb")
                h1_sb = work.tile([P, 2 * SW], dt.bfloat16, tag="h1",
                                  name="h1_sb")
                h2_sb = work.tile([P, 2 * SW], dt.bfloat16, tag="h2",
                                  name="h2_sb")
                # PE order fc1 -> h2 -> h1 -> st -> u hides every psum->sbuf
                # copy / tanh behind an independent matmul block.
                for blk in range(2):
                    din_ps = ps_g.tile([P, SW], dt.float32, tag="g",
                                       name="din_ps")
                    for k in range(2):
                        nc.tensor.matmul(
                            din_ps[:], lhsT=wfc1[k][:, blk * P:(blk + 1) * P],
                            rhs=ctxT_st[:, k * SW:(k + 1) * SW],
                            start=(k == 0), stop=(k == 1))
                    nc.vector.tensor_copy(din_sb[:, blk * SW:(blk + 1) * SW],
                                          din_ps[:])
                for blk in range(2):
                    h2_ps = ps_g.tile([P, SW], dt.float32, tag="g", name="h2_ps")
                    nc.tensor.matmul(h2_ps[:],
                                     lhsT=p2w1[:, blk * P:(blk + 1) * P],
                                     rhs=neT_st[:, SW:2 * SW])
                    if flags["p2b1"]:
                        nc.scalar.activation(h2_sb[:, blk * SW:(blk + 1) * SW],
                                             h2_ps[:], AF.Tanh,
                                             bias=osb["p2b1"][:, blk:blk + 1])
                    else:
                        nc.scalar.activation(h2_sb[:, blk * SW:(blk + 1) * SW],
                                             h2_ps[:], AF.Tanh)
                for blk in range(2):
                    h1_ps = ps_g.tile([P, SW], dt.float32, tag="g", name="h1_ps")
                    for k in range(2):
                        nc.tensor.matmul(
                            h1_ps[:], lhsT=p1w1[k][:, blk * P:(blk + 1) * P],
                            rhs=din_sb[:, k * SW:(k + 1) * SW],
                            start=(k == 0), stop=(k == 1))
                    if flags["p1b1"]:
                        nc.scalar.activation(h1_sb[:, blk * SW:(blk + 1) * SW],
                                             h1_ps[:], AF.Tanh,
                                             bias=osb["p1b1"][:, blk:blk + 1])
                    else:
                        nc.scalar.activation(h1_sb[:, blk * SW:(blk + 1) * SW],
                                             h1_ps[:], AF.Tanh)
                dump("d_din", din_sb[:])
                dump("d_h1", h1_sb[:])
                dump("d_h2", h2_sb[:])
                for s in range(ST):
                    st_ps = ps_m.tile([P, DM], dt.float32, tag="med",
                                      name="st_ps")
                    for k in range(2):
                        nc.tensor.matmul(
                            st_ps[:], lhsT=h2_sb[:, k * SW + s * P:
                                               k * SW + (s + 1) * P],
                            rhs=p2w2[k][:], start=(k == 0), stop=(k == 1))
                    if flags["p2b2"]:
                        nc.vector.tensor_add(st_ps[:], st_ps[:], osb["p2b2b"][:])
                    st_sl = st2_st[:, s * DM:(s + 1) * DM]
                    nc.vector.tensor_copy(st_sl, st_ps[:])
                    st6b = wsub.tile([P, 6], dt.float32, tag="st6b", name="st6b")
                    nc.vector.bn_stats(st6b[:], st_sl)
                    nc.vector.bn_aggr(mv_st[:, 4 * s + 2:4 * s + 4], st6b[:])
                for s in range(ST):
                    # u = dyn_in + h1 @ p1_w2.T : the residual is a matmul
                    # against identity (= transpose of din) accumulated into
                    # the same PSUM bank as the p1w2 matmuls.
                    u_ps = ps_m.tile([P, DM], dt.float32, tag="med", name="u_ps")
                    for k in range(2):
                        nc.tensor.matmul(
                            u_ps[:], lhsT=h1_sb[:, k * SW + s * P:
                                              k * SW + (s + 1) * P],
                            rhs=p1w2[k][:], start=(k == 0), stop=(k == 1))
                    nc.tensor.matmul(u_ps[:, 0:P],
                                     lhsT=din_sb[:, s * P:(s + 1) * P],
                                     rhs=ident[:], start=False, stop=False,
                                     skip_group_check=True)
                    nc.tensor.matmul(u_ps[:, P:2 * P],
                                     lhsT=din_sb[:, SW + s * P:SW + (s + 1) * P],
                                     rhs=ident[:], start=False, stop=True,
                                     skip_group_check=True)
                    u_sl = u2_st[:, s * DM:(s + 1) * DM]
                    nc.vector.tensor_copy(u_sl, u_ps[:])
                    if flags["p1b2"]:
                        nc.vector.tensor_add(u_sl, u_sl, osb["p1b2b"][:])
                    st6a = wsub.tile([P, 6], dt.float32, tag="st6a", name="st6a")
                    nc.vector.bn_stats(st6a[:], u_sl)
                    nc.vector.bn_aggr(mv_st[:, 4 * s:4 * s + 2], st6a[:])
                dump("d_u2", u2_st[:])
                dump("d_st2", st2_st[:])
                return mv_st, u2_st, st2_st

            pend = None
            fin_q = []
            nes = emit_gathers(0)
            head = emit_head(0, nes)
            nes = emit_gathers(1)
            for ss in range(nss):
                ctxT_st = emit_attn(ss, *head)
                if pend is not None:
                    fin_q.append(emit_tail(*pend))
                cur_neT = head[0]
                if ss + 1 < nss:
                    head = emit_head(ss + 1, nes)
                    if ss + 2 < nss:
                        nes = emit_gathers(ss + 2)
                mv_st, u2_st, st2_st = emit_mlp(ss, cur_neT, ctxT_st)
                if len(fin_q) >= 2:
                    emit_tail_fin(*fin_q.pop(0))
                pend = (ss, mv_st, u2_st, st2_st)
            fin_q.append(emit_tail(*pend))
            for f in fin_q:
                emit_tail_fin(*f)

            # ---- final divide + store
            r3 = res[:].rearrange("p (t k) -> p t k", k=2)
            rn = wsub.tile([GRP, n_sub], dt.float32, tag="rn", name="rn")
            rn3 = rn[:].rearrange("p (t o) -> p t o", o=1)
            nc.vector.reciprocal(rn3[:], r3[:, :, 1:2])
            orow = wsub.tile([GRP, n_sub], dt.float32, tag="orow", name="orow")
            orow3 = orow[:].rearrange("p (t o) -> p t o", o=1)
            nc.vector.tensor_tensor(out=orow3[:], in0=r3[:, :, 0:1], in1=rn3[:],
                                    op=OP.mult)
            nc.sync.dma_start(outp[:, :], orow[:])

    nc.finalize()
    return nc


# ----------------------------------------------------------------- entry
_NC_CACHE = {}


def kernel(**inputs):
    _install_ntff_hook()
    from concourse.bass_utils import run_bass_kernel_spmd

    n_sub = int(os.environ.get("KBENCH_NSUB", NSUB_FULL))
    consts = _prep_consts(inputs)
    flags = consts.pop("_flags")
    bcls = consts.pop("_bcls")

    key = (n_sub, tuple(sorted(flags.items())))
    if key not in _NC_CACHE:
        _NC_CACHE[key] = build_nc(flags, bcls, n_sub)
    nc = _NC_CACHE[key]

    x = np.asarray(inputs["x"]).astype(np.int32)
    in_maps = []
    for c in range(NCORES):
        xc = x[c * BC:(c + 1) * BC].reshape(-1)          # [16384]
        idxc = np.ascontiguousarray(
            xc[:n_sub * P].reshape(n_sub, P).T)          # [128, n_sub]
        m = {"idxc": idxc, "npmc": (idxc != 0).astype(np.float32)}
        m.update(consts)
        in_maps.append(m)

    trace = bool(int(os.environ.get("KBENCH_TRACE", "0")))
    res = run_bass_kernel_spmd(nc, in_maps, core_ids=list(range(NCORES)),
                               trace=trace)
    kernel._last_results = res

    out = np.zeros((B, 1), np.float32)
    for c in range(NCORES):
        oc = res.results[c]["outp"]                      # [8, n_sub]
        out[c * BC:c * BC + n_sub * GRP, 0] = oc.T.reshape(-1)
    return out
